# revision 8
# baseline (speedup 1.0000x reference)
"""Causal multi-head self-attention on 8 Trainium2 NeuronCores (Bass/Tile).

Problem (hardcoded): x [4, 2048, 1024] fp32, W_qkv [1024, 3072], b_qkv [3072],
W_out [1024, 1024], b_out [1024]. 16 heads, head_dim 64.

Sharding: core c = 2*b + g handles batch b (4 batches) and head group g
(8 heads). Host sums the two partial out-projections per batch (+ b_out).

v2 redesign vs baseline (everything f16 on the attention path):
 - AV matmul flipped: stationary = exp-score tile [kj 128, qi 128] (Ldweights
   is free), moving = v[+ones] [kj 128, 65] f16 -> out [qi 128, 65] PSUM.
   Cost 65 cyc/tile vs 512 unflipped; row-sums ride along as the ones column.
 - PSUM accumulation groups are bank-granular (start lazily zeroes the whole
   2KB zero region): a K=1 zeros-stationary matmul opens each AV bank with
   start=True, then the 4 qi-tile accumulators per head accumulate with
   start=False into disjoint 65-col sub-ranges of that bank.
 - Normalization is per-partition after the flip: DVE evicts av+sums to f16,
   DVE reciprocal on the sums column, then a stride-0 broadcast multiply.
   (These sit on latency-critical chains, so they run on DVE, not Pool:
   gpsimd's q7 launch + queue latency cost ~19us end-to-end.)
 - Causal mask: exp runs unmasked on the diagonal 128-wide strip, then DVE
   multiplies the diag block by a lower-triangular 0/1 f16 mask (saves the
   PE identity-matmul mask bias of the baseline).
 - attn comes out [qi, head dims]; PE f16 transposes (128 cyc each) restore
   [head dims, qi] for the output projection.
 - fp32 only in PSUM and the final host add; x/W/y move as f16/fp8.
 - qkv projection in fp8 e4m3 DoubleRow mode (0.5 cyc/row, K=256/pass) with
   3-term error compensation: x@W ~= xhi@Whi + xlo@Whi + (xhi/16)@(Wlo*16),
   hi/lo splits and DoubleRow interleaved layouts prepared on the host
   (the lo/hi16 scale factors keep the correction terms out of fp8
   subnormal range). Cuts qkv PE cycles 25% at ~2e-3 added relative error.
 - emission is software-pipelined and balance-tuned: qkv runs one qi-block
   ahead interleaved between attention pairs; outproj blocks are deferred
   into the exp-heaviest final iteration; W/x stream in first-use order.
"""
import ml_dtypes
import numpy as np

import concourse.bacc as bacc
import concourse.tile as tile
from concourse.tile import add_dep_helper
from concourse import mybir
from concourse.bass_utils import run_bass_kernel_spmd

B, L, D = 4, 2048, 1024
NKT0 = D // 128  # 8 contraction tiles
NH, HD = 16, 64
G = 8            # heads per core (group)
NP = G // 2      # head pairs per core
LC = 512         # qi block
KT = 128         # kj tile
NKJ = L // KT    # 16
F32 = mybir.dt.float32
F16 = mybir.dt.float16
F8 = mybir.dt.float8e4
DRM = mybir.MatmulPerfMode.DoubleRow
AF = mybir.ActivationFunctionType

E4 = ml_dtypes.float8_e4m3fn

_cache = {}


def _build():
    nc = bacc.Bacc("TRN2", target_bir_lowering=False, debug=False, num_devices=8)
    # fp8 e4m3 3-term error-compensated qkv operands (DoubleRow layout:
    # [part, pass, sub, cols], contraction row = pass*256 + sub*128 + part):
    # x @ W ~= xhi@Whi + xlo@Whi + (xhi/16)@(Wlo*16)
    x8h = nc.dram_tensor("x8h", [8, 128, 4, 2, 256], F8, kind="ExternalInput")
    x8l = nc.dram_tensor("x8l", [8, 128, 4, 2, 256], F8, kind="ExternalInput")
    x8h16 = nc.dram_tensor("x8h16", [8, 128, 4, 2, 256], F8,
                           kind="ExternalInput")
    W8 = nc.dram_tensor("W8", [128, 12, 2, 4, 2, 128], F8,
                        kind="ExternalInput")  # [.., m, (hi, lo16), ..]
    W_out_s = nc.dram_tensor("W_out_s", [G * HD, D], F16, kind="ExternalInput")
    tri = nc.dram_tensor("tri", [128, 128], F16, kind="ExternalInput")
    ident = nc.dram_tensor("ident", [128, 128], F16, kind="ExternalInput")
    yT = nc.dram_tensor("yT", [D, L], F16, kind="ExternalOutput")

    scale = float(1.0 / np.sqrt(HD))
    CH = 256              # qkv l-chunk
    NLC = L // LC         # 4 qi blocks of 512
    NM = (2 * G * HD) // 128   # 8 q+k col tiles of 128
    NKT = D // 128        # 8 contraction tiles
    VOFF = 2 * G * HD     # v column offset in W_in (1024)

    with tile.TileContext(nc) as tc:
        with tc.tile_pool(name="store", bufs=1) as store, \
             tc.tile_pool(name="qtp", bufs=2) as qtp, \
             tc.tile_pool(name="xtp", bufs=24) as xtp, \
             tc.tile_pool(name="expp", bufs=6) as expp, \
             tc.tile_pool(name="rawp", bufs=3) as rawp, \
             tc.tile_pool(name="recp", bufs=3) as recp, \
             tc.tile_pool(name="attnp", bufs=3) as attnp, \
             tc.tile_pool(name="attntp", bufs=4) as attntp, \
             tc.tile_pool(name="ytp", bufs=4) as ytp, \
             tc.tile_pool(name="ps", bufs=2, space="PSUM") as psp, \
             tc.tile_pool(name="scores", bufs=2, space="PSUM") as scores_p, \
             tc.tile_pool(name="av", bufs=1, space="PSUM") as av_p:
            kT_sb = store.tile([128, NP, L], F16)
            v_sb = store.tile([KT, NKJ, G, HD + 1], F16)
            W8_sb = store.tile([128, 12, 2, 4, 2, 128], F8)
            Wo_sb = store.tile([128, NP, D], F16)
            tri_sb = store.tile([128, 128], F16)
            id_sb = store.tile([128, 128], F16)
            zrow_sb = store.tile([1, 512], F16)

            nc.vector.memset(v_sb[:, :, :, HD:HD + 1], 1.0)
            nc.vector.memset(zrow_sb[:], 0.0)
            # Prefetch everything up front on the sync queue in first-use
            # order (HWDGE serializes DMAs anyway; a single queue gives exact
            # ordering control). The first m-group's operands stream as fine
            # pp-granular pieces so the first matmuls start ~2us earlier.
            xt_pre = [[xtp.tile([128, 4, 2, CH], F8, name=f"x{n}{c}",
                                tag="xt") for n in "hlm"] for c in range(8)]

            # chunk 0 / m-group (0,1) critical pieces, finely interleaved
            for pp in range(4):
                nc.sync.dma_start(out=xt_pre[0][0][:, pp], in_=x8h[0][:, pp])
                nc.sync.dma_start(out=W8_sb[:, 0, 0, pp], in_=W8[:, 0, 0, pp])
            nc.sync.dma_start(out=xt_pre[0][1][:], in_=x8l[0])
            nc.sync.dma_start(out=W8_sb[:, 0, 1], in_=W8[:, 0, 1])
            nc.sync.dma_start(out=xt_pre[0][2][:], in_=x8h16[0])
            nc.sync.dma_start(out=W8_sb[:, 1], in_=W8[:, 1])
            nc.sync.dma_start(out=tri_sb[:], in_=tri[:])
            nc.sync.dma_start(out=id_sb[:], in_=ident[:])
            # rest of W in qkv m-group consumption order, then x chunks
            for j in (4, 5, 2, 3, 6, 7):
                nc.sync.dma_start(out=W8_sb[:, j], in_=W8[:, j])
            for t, d in zip(xt_pre[1], (x8h, x8l, x8h16)):
                nc.sync.dma_start(out=t[:], in_=d[1])
            for j in (8, 9, 10, 11):
                nc.sync.dma_start(out=W8_sb[:, j], in_=W8[:, j])
            for c in range(2, 8):
                for t, d in zip(xt_pre[c], (x8h, x8l, x8h16)):
                    nc.sync.dma_start(out=t[:], in_=d[c])
                if c == 3:
                    nc.sync.dma_start(
                        out=Wo_sb[:],
                        in_=W_out_s.rearrange("(kt p) c -> p kt c", p=128))
            yT_r = yT.rearrange("(m p) l -> p m l", p=128)

            def qkv_chunk(c, qT_blk):
                l0 = c * CH
                half = (c % 2) * CH  # offset within the 512-wide qT_blk
                xh, xl, xm = xt_pre[c]
                terms = [(xh, 0), (xl, 0), (xm, 1)]

                def vsubs():
                    for sub in range(CH // KT):
                        ps = psp.tile([128, 512], F32, tag="ps")
                        opener = None
                        idx = 0
                        for vh in range(2):  # halves: the first starts once
                            # W m-tiles 8-9 land; explicit dep orders the
                            # second half's first write after the bank opener
                            for xa, Wa in terms:
                                for pp in range(4):
                                    mm = nc.tensor.matmul(
                                        ps[:, vh * 256:(vh + 1) * 256],
                                        xa[:, pp, :, sub * KT:(sub + 1) * KT],
                                        W8_sb[:, 8 + 2 * vh:10 + 2 * vh, Wa,
                                              pp].transpose([0, 2, 1, 3]),
                                        start=(idx == 0), stop=(idx == 23),
                                        skip_group_check=True,
                                        perf_mode=DRM)
                                    if idx == 0:
                                        opener = mm.ins
                                    elif idx == 12:
                                        add_dep_helper(mm.ins, opener,
                                                       reason="v bank order")
                                    idx += 1
                        nc.vector.tensor_copy(
                            out=v_sb[:, c * (CH // KT) + sub, :, 0:HD],
                            in_=ps[:].rearrange("p (h d) -> p h d", h=G))

                # two m-groups share one PSUM bank (pending-zero opens it,
                # an explicit dep orders the second group's first write) so
                # each pair evicts with a single strided DVE copy
                for mi, (ma, mb) in enumerate(((0, 1), (4, 5), (2, 3),
                                              (6, 7))):
                    ps = psp.tile([128, 2, CH], F32, tag="ps")
                    opener = None
                    for sl, m in enumerate((ma, mb)):
                        idx = 0
                        for xa, Wa in terms:
                            for pp in range(4):
                                mm = nc.tensor.matmul(
                                    ps[:, sl], W8_sb[:, m, Wa, pp], xa[:, pp],
                                    start=(sl == 0 and idx == 0),
                                    stop=(idx == 11),
                                    skip_group_check=True,
                                    perf_mode=DRM)
                                if sl == 0 and idx == 0:
                                    opener = mm.ins
                                elif sl == 1 and idx == 0:
                                    add_dep_helper(mm.ins, opener,
                                                   reason="qk bank order")
                                idx += 1
                    if ma < NP:
                        nc.vector.tensor_copy(
                            out=qT_blk[:, ma:ma + 2, half:half + CH],
                            in_=ps[:])
                    else:
                        nc.vector.tensor_copy(
                            out=kT_sb[:, ma - NP:ma - NP + 2, l0:l0 + CH],
                            in_=ps[:])
                    if mi == 3:
                        vsubs()

            def attention_pair(qb, pair, qT_blk, attnT_blk):
                n_t = (qb + 1) * (LC // KT)
                if True:
                    # av accumulators: bank h of 2, qi-tile oi slot at cols
                    # [oi*65, oi*65+65); col 64 collects the exp row-sums via
                    # the ones column of v. One K=1 zeros-stationary matmul
                    # per bank opens the (lazily-zeroed) accumulation group.
                    av = av_p.tile([128, 2, 512], F32, tag="av")
                    openers = {}

                    def avs(t, ex):
                        diag = t >= qb * (LC // KT)
                        o = t - qb * (LC // KT) if diag else 0
                        for oi in range(o, LC // KT):
                            last = t == qb * (LC // KT) + oi
                            for h in range(2):
                                # t=0, oi=0 opens the bank (start lazily
                                # zeroes the whole 2KB region); the other
                                # combos' first accumulates are explicitly
                                # ordered after it
                                mm = nc.tensor.matmul(
                                    av[:, h, oi * 65:oi * 65 + 65],
                                    ex[:, h, oi * KT:(oi + 1) * KT],
                                    v_sb[:, t, 2 * pair + h, :],
                                    start=(t == 0 and oi == 0), stop=last,
                                    skip_group_check=True)
                                if t == 0 and oi == 0:
                                    openers[h] = mm.ins
                                elif t == 0:
                                    add_dep_helper(mm.ins, openers[h],
                                                   reason="av bank open order")

                    # AV runs one tile behind scores/exp in emission order so
                    # a slow exp never head-of-line-blocks the next scores;
                    # the bank-opening zeroers sit after the first scores for
                    # the same reason (they wait on the previous pair's evict)
                    exs = {}
                    for t in range(n_t):
                        diag = t >= qb * (LC // KT)
                        o = t - qb * (LC // KT) if diag else 0
                        z = o * KT if diag else 0
                        sc = scores_p.tile([128, 1024], F32, tag="sc")
                        nc.tensor.matmul(
                            sc[:, z:LC],
                            kT_sb[0:64, pair, t * KT:(t + 1) * KT],
                            qT_blk[0:64, pair, z:LC], start=True, stop=True)
                        nc.tensor.matmul(
                            sc[:, LC + z:1024],
                            kT_sb[64:128, pair, t * KT:(t + 1) * KT],
                            qT_blk[64:128, pair, z:LC], start=True, stop=True)
                        ex = expp.tile([128, 2, LC], F16)
                        sc_v = sc[:].rearrange("p (h c) -> p h c", h=2)[:, :, z:LC]
                        nc.scalar.activation(ex[:, :, z:LC], sc_v, AF.Exp,
                                             scale=scale)
                        if diag:
                            # causal mask: zero the upper triangle of the
                            # diagonal 128-wide block post-exp on DVE (cheaper
                            # than PE identity-matmul bias; PE is the
                            # bottleneck engine)
                            nc.vector.tensor_mul(
                                ex[:, :, z:z + KT],
                                ex[:, :, z:z + KT],
                                tri_sb[:].unsqueeze(1).broadcast_to(
                                    [128, 2, KT]))
                        exs[t] = ex
                        if t >= 2:
                            avs(t - 2, exs.pop(t - 2))
                    for tt in range(max(0, n_t - 2), n_t):
                        avs(tt, exs.pop(tt))
                    # evict raw av+sums (frees the banks), reciprocal of the
                    # sums column, then Pool broadcast-multiply -> attn f16
                    raw = rawp.tile([128, 2, 4, 65], F16)
                    nc.vector.tensor_copy(
                        out=raw[:],
                        in_=av[:, :, 0:260].rearrange("p h (o e) -> p h o e",
                                                      o=4, e=65))
                    rec = recp.tile([128, 2, 4], F32)
                    nc.vector.reciprocal(out=rec[:], in_=raw[:, :, :, 64])
                    attn = attnp.tile([128, 4, 2, HD], F16)
                    nc.vector.tensor_mul(
                        attn[:],
                        raw[:, :, :, 0:HD].transpose([0, 2, 1, 3]),
                        rec[:].transpose([0, 2, 1]).unsqueeze(3).broadcast_to(
                            [128, 4, 2, HD]))
                    # PE f16 transposes: [qi 128, (2h x 64)] -> [hd 128, qi]
                    pst = psp.tile([128, 4, 128], F16, tag="ps")
                    for oi in range(4):
                        nc.tensor.transpose(pst[:, oi, :], attn[:, oi, :, :],
                                            id_sb[:])
                    nc.vector.tensor_copy(out=attnT_blk[:, pair, :],
                                          in_=pst[:].rearrange("p o c -> p (o c)"))

            def outproj_ms(qb, ms, attnT_blk, evict_act=False):
                l0 = qb * LC
                for m in ms:
                    ps = psp.tile([128, 512], F32, tag="ps")
                    for kt in range(NP):
                        nc.tensor.matmul(
                            ps[:], Wo_sb[:, kt, m * 128:(m + 1) * 128],
                            attnT_blk[:, kt, :],
                            start=(kt == 0), stop=(kt == NP - 1))
                    yt = ytp.tile([128, 512], F16)
                    if evict_act and m % 2:
                        nc.scalar.copy(yt[:], ps[:])
                    else:
                        nc.vector.tensor_copy(out=yt[:], in_=ps[:])
                    nc.scalar.dma_start(out=yT_r[:, m, l0:l0 + LC], in_=yt[:])

            # Software pipeline: qkv runs one qi-block ahead of attention,
            # interleaved between attention pairs so the scheduler always has
            # independent PE work during the ACT-paced exp stretches;
            # outproj(qb-1) fills the remaining slots.
            attnT_blks = {}
            qT_blks = {0: qtp.tile([128, NP, LC], F16, name="qT0", tag="qT")}
            qkv_chunk(0, qT_blks[0])
            qkv_chunk(1, qT_blks[0])
            # fill placement balances PE work against the linearly-growing
            # exp (ACT) load per iteration: chunk 7 and most outproj blocks
            # are deferred into the ACT-heaviest iteration 3 (chunk 7 only
            # gates the last two kj tiles of attention(3), which run last)
            fill_sched = {0: [("qkv", 2), ("qkv", 3)],
                          1: [("qkv", 4), ("qkv", 5),
                              ("out", 0, range(0, 4)),
                              ("out", 0, range(4, 8))],
                          2: [("qkv", 6), ("qkv", 7),
                              ("out", 1, range(0, 4))],
                          3: [("out", 1, range(4, 8)),
                              ("out", 2, range(0, 4)),
                              ("out", 2, range(4, 8))]}
            qkv_dest = {c: (c // 2) for c in range(8)}
            for qb in range(NLC):
                if qb + 1 < NLC:
                    qT_blks[qb + 1] = qtp.tile([128, NP, LC], F16,
                                               name=f"qT{qb + 1}", tag="qT")
                attnT_blks[qb] = attntp.tile([128, NP, LC], F16,
                                             name=f"attnT{qb}", tag="attnT")
                fills = list(fill_sched.get(qb, []))
                if len(fills) == 2:
                    fills = [fills[0], None, fills[1], None]
                for pair in range(NP):
                    attention_pair(qb, pair, qT_blks[qb], attnT_blks[qb])
                    for f in (fills[pair:pair + 1] if pair < 3
                              else fills[pair:]):
                        if f is None:
                            continue
                        # demote fill priority: on scheduler ties the
                        # latency-critical attention chain should win over
                        # bulk fill work
                        save = tc.cur_priority
                        tc.cur_priority = save + (500000 if f[0] == "qkv"
                                                  else 100000)
                        if f[0] == "qkv":
                            qkv_chunk(f[1], qT_blks[qkv_dest[f[1]]])
                        else:
                            outproj_ms(f[1], f[2], attnT_blks[f[1]])
                        tc.cur_priority = save
            outproj_ms(NLC - 1, range(8), attnT_blks[NLC - 1])
    nc.compile()
    return nc


def _make_tri():
    # multiplicative causal keep-mask for the diagonal strip, applied on DVE
    # post-exp: tri[kj_local, qi_local] = 1 where kj <= qi else 0
    keep = np.tril(np.ones((128, 128), np.float32)).T
    return keep.astype(np.float16)


def _make_ident():
    return np.eye(128, dtype=np.float16)


def kernel(x, W_qkv, b_qkv, W_out, b_out, _trace=False, _trace_kwargs=None):
    x = np.ascontiguousarray(x, dtype=np.float32)
    W_qkv = np.asarray(W_qkv, dtype=np.float32)
    b_qkv = np.asarray(b_qkv, dtype=np.float32)
    W_out = np.asarray(W_out, dtype=np.float32)
    b_out = np.asarray(b_out, dtype=np.float32)
    assert np.all(b_qkv == 0.0), "nonzero b_qkv not supported by this kernel"

    if "nc" not in _cache:
        _cache["nc"] = _build()
    nc = _cache["nc"]

    tri = _make_tri()
    ident = _make_ident()
    Wq, Wk, Wv = W_qkv[:, 0:D], W_qkv[:, D:2 * D], W_qkv[:, 2 * D:3 * D]

    in_maps = []
    for c in range(8):
        b, g = divmod(c, 2)
        cols = slice(g * G * HD, (g + 1) * G * HD)
        W_in = np.concatenate([Wq[:, cols], Wk[:, cols], Wv[:, cols]], axis=1)

        def packw(A):  # [1024, 1536] -> [128, 12 m, 4 pass, 2 sub, 128]
            return np.ascontiguousarray(
                A.reshape(4, 2, 128, 12, 128).transpose(2, 3, 0, 1, 4))

        def packx(A):  # [1024, 2048] -> [8 chunk, 128, 4 pass, 2 sub, 256]
            t = A.reshape(4, 2, 128, L).transpose(2, 0, 1, 3)
            return np.ascontiguousarray(
                t.reshape(128, 4, 2, 8, 256).transpose(3, 0, 1, 2, 4))

        Whi = W_in.astype(E4)
        Wlo16 = ((W_in - Whi.astype(np.float32)) * 16).astype(E4)
        xTb = x[b].T
        xhi = xTb.astype(E4)
        xlo = (xTb - xhi.astype(np.float32)).astype(E4)
        xhi16 = (xhi.astype(np.float32) / 16).astype(E4)
        in_maps.append({
            "x8h": packx(xhi), "x8l": packx(xlo), "x8h16": packx(xhi16),
            "W8": np.ascontiguousarray(
                np.stack([packw(Whi), packw(Wlo16)], axis=2)),
            "W_out_s": np.ascontiguousarray(W_out[cols, :].astype(np.float16)),
            "tri": tri,
            "ident": ident,
        })

    kw = {}
    if _trace:
        kw["trace"] = True
        kw.update(_trace_kwargs or {})
    res = run_bass_kernel_spmd(nc, in_maps, list(range(8)), **kw)

    out = np.empty((B, L, D), dtype=np.float32)
    for b in range(B):
        yT = (res.results[2 * b]["yT"].astype(np.float32)
              + res.results[2 * b + 1]["yT"].astype(np.float32))
        out[b] = yT.T + b_out
    if _trace:
        _cache["last_result"] = res
    return out

